# revision 16
# baseline (speedup 1.0000x reference)
"""Trainium2 Bass kernel for nn_GAT_54546084659384 (gnn_message_passing).

Strategy: pure data-parallel over the batch dim B=1024 across 8 NeuronCores
(128 batches/core).  All matmuls in bf16 (fp32 PSUM accumulate).  The big
MLP weight (W_mlp, 256MB fp32) is pre-transposed + cast to bf16 on the host
and streamed from HBM on every core (~128MB/core, the HBM roofline term).

kernel(**inputs) takes the FULL unsharded inputs and returns the full
(out, e2, m) tuple matching reference.reference().
"""

import os
import sys

sys.path.insert(0, "/opt/trn_rl_repo")

import numpy as np
import ml_dtypes

BF16 = ml_dtypes.bfloat16

B, N, F_IN, F_OUT = 1024, 16, 1024, 4096
NCORES = 8
BL = B // NCORES          # 128 batches per core
ROWS = BL * N             # 2048 rows per core
NG = BL // 8              # 16 groups of 8 batches
GB = 8                    # batches per group

LAST_EXEC_NS = None
_CACHE = {}


def _build_program(variant=4):
    import concourse.bacc as bacc
    import concourse.bass as bass
    import concourse.tile as tile
    import concourse.mybir as mybir
    from contextlib import ExitStack

    dt = mybir.dt
    AF = mybir.ActivationFunctionType
    ALU = mybir.AluOpType

    nc = bacc.Bacc("TRN2", target_bir_lowering=False, debug=False,
                   num_devices=NCORES)

    xn_d = nc.dram_tensor("xn", [ROWS, F_IN], dt.bfloat16, kind="ExternalInput").ap()
    xt_d = nc.dram_tensor("xt", [F_IN, ROWS], dt.bfloat16, kind="ExternalInput").ap()
    mat_d = nc.dram_tensor("mat", [ROWS, N], dt.bfloat16, kind="ExternalInput").ap()
    w1t_d = nc.dram_tensor("w1t", [F_IN, F_IN], dt.bfloat16, kind="ExternalInput").ap()
    b1t_d = nc.dram_tensor("b1t", [128, 8], dt.float32, kind="ExternalInput").ap()
    wmt_d = nc.dram_tensor("wmt", [N * F_IN, F_OUT], dt.bfloat16, kind="ExternalInput").ap()
    bm_d = nc.dram_tensor("bm", [1, F_OUT], dt.bfloat16, kind="ExternalInput").ap()
    mask_d = nc.dram_tensor("mask", [N, 128], dt.float32, kind="ExternalInput").ap()
    dmask_d = nc.dram_tensor("dmask", [N, 128], dt.bfloat16, kind="ExternalInput").ap()
    onesv_d = nc.dram_tensor("onesv", [128, 1], dt.bfloat16, kind="ExternalInput").ap()
    ones1_d = nc.dram_tensor("ones1", [1, 128], dt.bfloat16, kind="ExternalInput").ap()

    out_d = nc.dram_tensor("out", [BL, F_OUT], dt.float32, kind="ExternalOutput").ap()
    e2_d = nc.dram_tensor("e2", [ROWS, N], dt.float32, kind="ExternalOutput").ap()

    F32 = dt.float32
    B16 = dt.bfloat16

    with tile.TileContext(nc) as tc, ExitStack() as ctx:
        const = ctx.enter_context(tc.tile_pool(name="const", bufs=1))
        xtp = ctx.enter_context(tc.tile_pool(name="xtp", bufs=1))
        xnp = ctx.enter_context(tc.tile_pool(name="xnp", bufs=3))
        wgp = ctx.enter_context(tc.tile_pool(name="wgp", bufs=1))
        atp = ctx.enter_context(tc.tile_pool(name="atp", bufs=1))
        wmp = ctx.enter_context(tc.tile_pool(name="wmp", bufs=4))
        outp = ctx.enter_context(tc.tile_pool(name="outp", bufs=2))

        # ---- constants ----
        b1t_t = const.tile([128, 8], F32, tag="b1t")
        nc.sync.dma_start(b1t_t[:], b1t_d[:])
        mask_t = const.tile([N, 128], F32, tag="mask")
        nc.sync.dma_start(mask_t[:], mask_d[:])
        dmask_t = const.tile([N, 128], B16, tag="dmask")
        nc.sync.dma_start(dmask_t[:], dmask_d[:])
        onesv_t = const.tile([128, 1], B16, tag="onesv")
        nc.sync.dma_start(onesv_t[:], onesv_d[:])
        ones1_t = const.tile([1, 128], B16, tag="ones1")
        nc.sync.dma_start(ones1_t[:], ones1_d[:])
        # 1/|x_row| for rows (b,i) [first 2048] and 1/|wg_row| [next 2048],
        # free-major so slices can feed outer-product matmuls.
        rn_all = const.tile([1, 2 * ROWS], F32, tag="rnall")
        rn16 = const.tile([1, 2 * ROWS], B16, tag="rn16")

        # x transposed (feature-on-partition), 8 chunks of [128, 2048]
        xt_t = []
        for kc in range(8):
            t = xtp.tile([128, ROWS], B16, tag=f"xt{kc}", name=f"xt{kc}")
            nc.sync.dma_start(t[:], xt_d[kc * 128:(kc + 1) * 128, :])
            xt_t.append(t)

        # wg transposed (kout-on-partition), filled in phase 0
        wgT_t = [wgp.tile([128, ROWS], B16, tag=f"wg{ko}", name=f"wg{ko}")
                 for ko in range(8)]

        # attn transposed, tile fs: [128 f, (i,b)] -> f' = i*1024 + fs*128 + p
        at_t = [atp.tile([128, N * BL], B16, tag=f"at{fs}", name=f"at{fs}")
                for fs in range(8)]
        if variant < 4:
            for fs in range(8):
                nc.vector.memset(at_t[fs][:], 0.0)

        # ================= phase 0: wg + norms =================
        with ExitStack() as p0:
            w1p = p0.enter_context(tc.tile_pool(name="w1p", bufs=1))
            sb0 = p0.enter_context(tc.tile_pool(name="sb0", bufs=3))
            ps_wg = p0.enter_context(
                tc.tile_pool(name="ps_wg", bufs=2, space="PSUM"))
            ps_n = p0.enter_context(
                tc.tile_pool(name="ps_n", bufs=2, space="PSUM"))

            w1t_t = []
            for kc in range(8):
                t = w1p.tile([128, F_IN], B16, tag=f"w1t{kc}", name=f"w1t{kc}")
                nc.sync.dma_start(t[:], w1t_d[kc * 128:(kc + 1) * 128, :])
                w1t_t.append(t)

            # --- x norms: |x_(b,i)|^2 = colsum over k of xt^2 ---
            for rb in range(4):
                sl = slice(rb * 512, (rb + 1) * 512)
                pn = ps_n.tile([1, 512], F32, tag="pn")
                for kc in range(8):
                    sq = sb0.tile([128, 512], B16, tag="sq")
                    nc.scalar.activation(sq[:], xt_t[kc][:, sl], AF.Square)
                    nc.tensor.matmul(pn[:], onesv_t[:], sq[:],
                                     start=(kc == 0), stop=(kc == 7))
                s = sb0.tile([1, 512], F32, tag="s")
                nc.scalar.sqrt(s[:], pn[:])
                nc.vector.reciprocal(rn_all[0:1, sl], s[:])
                nc.vector.tensor_copy(rn16[0:1, sl], rn_all[0:1, sl])

            # --- wg = W1 @ x^T + b1 (stored transposed) + wg norms ---
            for rb in range(4):
                sl = slice(rb * 512, (rb + 1) * 512)
                pnw = ps_n.tile([1, 512], F32, tag="pnw")
                for ko in range(8):
                    pw = ps_wg.tile([128, 512], F32, tag="pw")
                    for ki in range(8):
                        nc.tensor.matmul(
                            pw[:],
                            w1t_t[ki][:, ko * 128:(ko + 1) * 128],
                            xt_t[ki][:, sl],
                            start=(ki == 0), stop=(ki == 7))
                    nc.vector.tensor_scalar_add(
                        wgT_t[ko][:, sl], pw[:], b1t_t[:, ko:ko + 1])
                    sqw = sb0.tile([128, 512], B16, tag="sqw")
                    nc.scalar.activation(sqw[:], pw[:], AF.Square,
                                         bias=b1t_t[:, ko:ko + 1])
                    nc.tensor.matmul(pnw[:], onesv_t[:], sqw[:],
                                     start=(ko == 0), stop=(ko == 7))
                s = sb0.tile([1, 512], F32, tag="s")
                nc.scalar.sqrt(s[:], pnw[:])
                wsl0 = slice(ROWS + rb * 512, ROWS + (rb + 1) * 512)
                nc.vector.reciprocal(rn_all[0:1, wsl0], s[:])
                nc.vector.tensor_copy(rn16[0:1, wsl0], rn_all[0:1, wsl0])

        # ================= phase 1: per-group graph attention =================
        # All small per-batch 16x16 matrices live as [16 partitions,
        # (batch, 16) free] tiles: every matmul runs at partition base 0
        # (single PE tile config) which is the safe/fast path on HW.
        with ExitStack() as p1:
            sb1 = p1.enter_context(tc.tile_pool(name="sb1", bufs=3))
            xnp1 = p1.enter_context(tc.tile_pool(name="xnp1", bufs=2))
            ps_sm = p1.enter_context(
                tc.tile_pool(name="ps_sm", bufs=6, space="PSUM"))
            ps_at = p1.enter_context(
                tc.tile_pool(name="ps_at", bufs=2, space="PSUM"))

            for g in range(NG if variant >= 2 else 0):
                rsl = slice(g * 128, (g + 1) * 128)

                # x natural, batch-major free packing: xn16[r, b*1024+f]
                xn16 = xnp1.tile([N, GB * F_IN], B16, tag="xn16", name="xn16")
                nc.sync.dma_start(
                    xn16[:].rearrange("p (b f) -> p b f", b=GB),
                    xn_d[rsl, :].rearrange("(b r) f -> r b f", r=N))

                def csl(b):
                    gb = g * GB + b
                    return slice(gb * 16, (gb + 1) * 16)

                # raw scores, both orientations
                pET = ps_sm.tile([N, 128], F32, tag="sm", name="pET",
                                 padded_shape=[128, 512])
                pPN = ps_sm.tile([N, 128], F32, tag="sm", name="pPN",
                                 padded_shape=[128, 512])
                for b in range(GB):
                    bs = slice(b * 16, (b + 1) * 16)
                    c = csl(b)
                    for kc in range(8):
                        nc.tensor.matmul(pET[:, bs], wgT_t[kc][:, c],
                                         xt_t[kc][:, c],
                                         start=(kc == 0), stop=(kc == 7))
                    for kc in range(8):
                        nc.tensor.matmul(pPN[:, bs], xt_t[kc][:, c],
                                         wgT_t[kc][:, c],
                                         start=(kc == 0), stop=(kc == 7))

                if variant == 2:
                    ets = sb1.tile([N, 128], F32, tag="ets", name="ets")
                    nc.vector.tensor_copy(ets[:], pET[:])
                    nc.vector.tensor_tensor(ets[:], ets[:], pPN[:], op=ALU.add)
                    nc.sync.dma_start(
                        e2_d[rsl, :].rearrange("(b i) j -> i b j", i=N),
                        ets[:].rearrange("p (b j) -> p b j", b=GB))
                    continue

                # outer-product norm scalers
                pM1 = ps_sm.tile([N, 128], F32, tag="sm", name="pM1",
                                 padded_shape=[128, 512])
                pM2 = ps_sm.tile([N, 128], F32, tag="sm", name="pM2",
                                 padded_shape=[128, 512])
                for b in range(GB):
                    bs = slice(b * 16, (b + 1) * 16)
                    c = csl(b)
                    xs = slice(c.start, c.stop)
                    ws = slice(ROWS + c.start, ROWS + c.stop)
                    nc.tensor.matmul(pM1[:, bs], rn16[0:1, ws],
                                     rn16[0:1, xs], start=True, stop=True)
                    nc.tensor.matmul(pM2[:, bs], rn16[0:1, xs],
                                     rn16[0:1, ws], start=True, stop=True)

                # scale + diagonal boost:  E = raw * rn_outer * mask
                m1s = sb1.tile([N, 128], F32, tag="m1s", name="m1s")
                nc.vector.tensor_tensor(m1s[:], pM1[:], mask_t[:], op=ALU.mult)
                m2s = sb1.tile([N, 128], F32, tag="m2s", name="m2s")
                nc.vector.tensor_tensor(m2s[:], pM2[:], mask_t[:], op=ALU.mult)
                ets = sb1.tile([N, 128], F32, tag="ets", name="ets")
                nc.vector.tensor_tensor(ets[:], pET[:], m1s[:], op=ALU.mult)
                enb = sb1.tile([N, 128], B16, tag="enb", name="enb")
                nc.vector.tensor_tensor(enb[:], pPN[:], m2s[:], op=ALU.mult)

                # softmax over i (per 16-col segment)
                sexp = sb1.tile([N, 128], F32, tag="sexp", name="sexp")
                nc.scalar.activation(sexp[:], ets[:], AF.Exp)
                ssum = sb1.tile([N, GB], F32, tag="ssum", name="ssum")
                nc.vector.tensor_reduce(
                    ssum[:], sexp[:].rearrange("p (b i) -> p b i", b=GB),
                    axis=mybir.AxisListType.X, op=ALU.add)
                rs = sb1.tile([N, GB], F32, tag="rs", name="rs")
                nc.vector.reciprocal(rs[:], ssum[:])
                stb = sb1.tile([N, 128], B16, tag="stb", name="stb")
                for b in range(GB):
                    bs = slice(b * 16, (b + 1) * 16)
                    nc.vector.tensor_scalar_mul(stb[:, bs], sexp[:, bs],
                                                rs[:, b:b + 1])

                # message passing
                mt = sb1.tile([N, 128], B16, tag="mt", name="mt")
                nc.sync.dma_start(
                    mt[:].rearrange("p (b k) -> p b k", b=GB),
                    mat_d[rsl, :].rearrange("(b t) k -> t b k", t=N))
                dscr = sb1.tile([N, 128], B16, tag="dscr", name="dscr")
                nc.vector.tensor_tensor(dscr[:], mt[:], dmask_t[:], op=ALU.mult)
                dv = sb1.tile([N, GB], F32, tag="dv", name="dv")
                nc.vector.tensor_reduce(
                    dv[:], dscr[:].rearrange("p (b k) -> p b k", b=GB),
                    axis=mybir.AxisListType.X, op=ALU.add)

                pFI = ps_sm.tile([N, 128], F32, tag="sm", name="pFI",
                                 padded_shape=[128, 512])
                for b in range(GB):
                    bs = slice(b * 16, (b + 1) * 16)
                    nc.tensor.matmul(pFI[:, bs], mt[:, bs], stb[:, bs],
                                     start=True, stop=True)
                fit = sb1.tile([N, 128], B16, tag="fit", name="fit")
                nc.vector.tensor_copy(fit[:], pFI[:])
                fjt = sb1.tile([N, 128], B16, tag="fjt", name="fjt")
                for b in range(GB):
                    bs = slice(b * 16, (b + 1) * 16)
                    nc.vector.tensor_scalar_mul(fjt[:, bs], stb[:, bs],
                                                dv[:, b:b + 1])

                pE2 = ps_sm.tile([N, 128], F32, tag="sm", name="pE2",
                                 padded_shape=[128, 512])
                pE2r = ps_sm.tile([N, 128], F32, tag="sm", name="pE2r",
                                  padded_shape=[128, 512])
                for b in range(GB):
                    bs = slice(b * 16, (b + 1) * 16)
                    nc.tensor.matmul(pE2[:, bs], fit[:, bs], fjt[:, bs],
                                     start=True, stop=True)
                    nc.tensor.matmul(pE2r[:, bs], fjt[:, bs], fit[:, bs],
                                     start=True, stop=True)
                cpr = sb1.tile([N, 128], F32, tag="cpr", name="cpr")
                nc.vector.tensor_scalar(cpr[:], pE2r[:], 0.5, None,
                                        op0=ALU.mult)
                e2s = sb1.tile([N, 128], F32, tag="e2s", name="e2s")
                nc.vector.scalar_tensor_tensor(
                    e2s[:], pE2[:], 0.5, cpr[:], op0=ALU.mult, op1=ALU.add)
                nc.sync.dma_start(
                    e2_d[rsl, :].rearrange("(b i) j -> i b j", i=N),
                    e2s[:].rearrange("p (b j) -> p b j", b=GB))

                # attn^T = tanh(X^T @ E), scattered into at_t
                if variant < 4:
                    continue
                for fs in range(8):
                    pa = ps_at.tile([128, 128], F32, tag="pa", name="pa",
                                    padded_shape=[128, 512])
                    for b in range(GB):
                        bs = slice(b * 16, (b + 1) * 16)
                        nc.tensor.matmul(
                            pa[:, bs],
                            xn16[:, b * F_IN + fs * 128:b * F_IN + (fs + 1) * 128],
                            enb[:, bs], start=True, stop=True)
                    src_ap = pa[:].rearrange("p (b i) -> p i b", b=GB)
                    dst_ap = at_t[fs][:].rearrange("p (i b) -> p i b", i=N)[
                        :, :, g * GB:(g + 1) * GB]
                    nc.scalar.activation(dst_ap, src_ap, AF.Tanh)

        # ================= phase 2: big MLP =================
        with ExitStack() as p2:
            ps2 = p2.enter_context(
                tc.tile_pool(name="ps2", bufs=1, space="PSUM"))
            bmp = p2.enter_context(tc.tile_pool(name="bmp", bufs=1))
            bm_t = bmp.tile([1, F_OUT], B16, tag="bm")
            nc.sync.dma_start(bm_t[:], bm_d[:])
            pso = [ps2.tile([128, 512], F32, tag=f"o{oc}", name=f"pso{oc}")
                   for oc in range(8)]
            for fcg in range(128):
                i_, fs = divmod(fcg, 8)
                wm = wmp.tile([128, F_OUT], B16, tag="wm")
                for h in range(2):
                    nc.sync.dma_start(
                        wm[:, h * 2048:(h + 1) * 2048],
                        wmt_d[fcg * 128:(fcg + 1) * 128, h * 2048:(h + 1) * 2048])
                lhs = at_t[fs][:, i_ * 128:(i_ + 1) * 128]
                for oc in range(8):
                    nc.tensor.matmul(pso[oc][:], lhs,
                                     wm[:, oc * 512:(oc + 1) * 512],
                                     start=(fcg == 0), stop=False)
            for oc in range(8):
                osl = slice(oc * 512, (oc + 1) * 512)
                nc.tensor.matmul(pso[oc][:], ones1_t[:], bm_t[0:1, osl],
                                 start=False, stop=True)
                ot = outp.tile([128, 512], F32, tag="ot")
                nc.vector.tensor_copy(ot[:], pso[oc][:])
                nc.sync.dma_start(out_d[:, osl], ot[:])

    nc.compile()
    return nc


def _get_program():
    variant = int(os.environ.get("KERNEL_VARIANT", "4"))
    key = f"nc{variant}"
    if key not in _CACHE:
        _CACHE[key] = _build_program(variant)
    return _CACHE[key]


def _prep_inputs(x, matrix, W1, b1, W_mlp, b_mlp, a_param):
    x = np.asarray(x, dtype=np.float32)
    matrix = np.asarray(matrix, dtype=np.float32)
    W1 = np.asarray(W1, dtype=np.float32)
    b1 = np.asarray(b1, dtype=np.float32)
    W_mlp = np.asarray(W_mlp, dtype=np.float32)
    b_mlp = np.asarray(b_mlp, dtype=np.float32)
    a = float(a_param)

    xr = x.reshape(NCORES, ROWS, F_IN)
    xn_h = xr.astype(BF16)
    xt_h = np.ascontiguousarray(xr.transpose(0, 2, 1)).astype(BF16)
    mat_h = matrix.reshape(NCORES, ROWS, N).astype(BF16)
    w1t_h = np.ascontiguousarray(W1.T).astype(BF16)
    b1t_h = np.ascontiguousarray(b1.reshape(8, 128).T)
    wmt_h = np.ascontiguousarray(W_mlp.astype(BF16).T)
    bm_h = b_mlp.reshape(1, F_OUT).astype(BF16)

    eye16 = np.eye(16, dtype=np.float32)
    eye_t = np.tile(eye16, (1, 8))                  # [16, 128]
    mask_h = (1.0 + a * eye_t).astype(np.float32)
    dmask_h = eye_t.astype(BF16)
    onesv_h = np.ones((128, 1), BF16)
    ones1_h = np.ones((1, 128), BF16)

    in_maps = []
    for c in range(NCORES):
        in_maps.append({
            "xn": np.ascontiguousarray(xn_h[c]),
            "xt": np.ascontiguousarray(xt_h[c]),
            "mat": np.ascontiguousarray(mat_h[c]),
            "w1t": w1t_h, "b1t": b1t_h, "wmt": wmt_h, "bm": bm_h,
            "mask": mask_h, "dmask": dmask_h,
            "onesv": onesv_h, "ones1": ones1_h,
        })
    return in_maps


def _install_ntff_hook():
    """Provide antenv.axon_hooks if the image lacks it (mirrors trn_boot)."""
    import types
    import ctypes
    import contextlib

    try:
        from antenv.axon_hooks import get_axon_ntff_profile_hook  # noqa: F401
        return
    except ImportError:
        pass

    so_path = "/opt/axon/libaxon_pjrt.so"
    if not os.path.exists(so_path):
        return
    lib = ctypes.CDLL(so_path)
    if not hasattr(lib, "axon_start_nrt_profile"):
        return
    lib.axon_start_nrt_profile.argtypes = [
        ctypes.POINTER(ctypes.c_int64), ctypes.c_size_t]
    lib.axon_start_nrt_profile.restype = ctypes.c_int64
    lib.axon_stop_nrt_profile.argtypes = [ctypes.c_char_p]
    lib.axon_stop_nrt_profile.restype = ctypes.c_int64

    @contextlib.contextmanager
    def _hook(output_dir, device_ids):
        import jax
        jax.devices()
        if device_ids:
            ids = (ctypes.c_int64 * len(device_ids))(*device_ids)
            rc = lib.axon_start_nrt_profile(ids, len(device_ids))
        else:
            rc = lib.axon_start_nrt_profile(None, 0)
        if rc != 0:
            raise RuntimeError(f"axon_start_nrt_profile rc={rc}")
        try:
            yield
        finally:
            n = lib.axon_stop_nrt_profile(str(output_dir).encode())
            print(f"profile: {n} file(s) written to {output_dir}",
                  file=sys.stderr)

    import antenv
    mod = types.ModuleType("antenv.axon_hooks")
    mod.get_axon_ntff_profile_hook = lambda: _hook
    mod.set_axon_ntff_profile_hook = lambda h: None
    sys.modules["antenv.axon_hooks"] = mod
    antenv.axon_hooks = mod


def kernel(x, matrix, W1, b1, W_mlp, b_mlp, a_param):
    global LAST_EXEC_NS
    from concourse import bass_utils

    nc = _get_program()
    in_maps = _prep_inputs(x, matrix, W1, b1, W_mlp, b_mlp, a_param)

    trace = os.environ.get("KERNEL_TRACE") == "1"
    if trace:
        _install_ntff_hook()
        try:
            res = bass_utils.run_bass_kernel_spmd(
                nc, in_maps, core_ids=list(range(NCORES)), trace=True,
                tmpdir=os.environ.get("KERNEL_TRACE_DIR"))
        except Exception as e:
            print(f"traced run failed ({e!r}); retrying untraced",
                  file=sys.stderr)
            res = bass_utils.run_bass_kernel_spmd(
                nc, in_maps, core_ids=list(range(NCORES)), trace=False)
    else:
        res = bass_utils.run_bass_kernel_spmd(
            nc, in_maps, core_ids=list(range(NCORES)), trace=False)
    LAST_EXEC_NS = res.exec_time_ns

    outs = res.results
    out_b = np.concatenate([np.asarray(outs[c]["out"]) for c in range(NCORES)], axis=0)
    e2 = np.concatenate([np.asarray(outs[c]["e2"]) for c in range(NCORES)], axis=0)
    e2 = np.ascontiguousarray(e2.reshape(B, N, N, 1))
    out_full = np.ascontiguousarray(
        np.broadcast_to(out_b[:, None, :], (B, N, F_OUT)))
    m = np.asarray(matrix, dtype=np.float32).reshape(B, N, N, 1)
    return out_full, e2, m


# revision 18
# speedup vs baseline: 1.0727x; 1.0727x over previous
"""Trainium2 Bass kernel for nn_GAT_54546084659384 (gnn_message_passing).

Strategy: pure data-parallel over the batch dim B=1024 across 8 NeuronCores
(128 batches/core).  All matmuls in bf16 (fp32 PSUM accumulate).  The big
MLP weight (W_mlp, 256MB fp32) is pre-transposed + cast to bf16 on the host
and streamed from HBM on every core (~128MB/core, the HBM roofline term).

kernel(**inputs) takes the FULL unsharded inputs and returns the full
(out, e2, m) tuple matching reference.reference().
"""

import os
import sys

sys.path.insert(0, "/opt/trn_rl_repo")

import numpy as np
import ml_dtypes

BF16 = ml_dtypes.bfloat16

B, N, F_IN, F_OUT = 1024, 16, 1024, 4096
NCORES = 8
BL = B // NCORES          # 128 batches per core
ROWS = BL * N             # 2048 rows per core
NG = BL // 8              # 16 groups of 8 batches
GB = 8                    # batches per group

LAST_EXEC_NS = None
_CACHE = {}


def _build_program(variant=4):
    import concourse.bacc as bacc
    import concourse.bass as bass
    import concourse.tile as tile
    import concourse.mybir as mybir
    from contextlib import ExitStack

    dt = mybir.dt
    AF = mybir.ActivationFunctionType
    ALU = mybir.AluOpType

    nc = bacc.Bacc("TRN2", target_bir_lowering=False, debug=False,
                   num_devices=NCORES)

    xn_d = nc.dram_tensor("xn", [ROWS, F_IN], dt.bfloat16, kind="ExternalInput").ap()
    xt_d = nc.dram_tensor("xt", [F_IN, ROWS], dt.bfloat16, kind="ExternalInput").ap()
    mat_d = nc.dram_tensor("mat", [ROWS, N], dt.bfloat16, kind="ExternalInput").ap()
    w1t_d = nc.dram_tensor("w1t", [F_IN, F_IN], dt.bfloat16, kind="ExternalInput").ap()
    b1t_d = nc.dram_tensor("b1t", [128, 8], dt.float32, kind="ExternalInput").ap()
    wmt_d = nc.dram_tensor("wmt", [N * F_IN, F_OUT], dt.bfloat16, kind="ExternalInput").ap()
    bm_d = nc.dram_tensor("bm", [1, F_OUT], dt.bfloat16, kind="ExternalInput").ap()
    mask_d = nc.dram_tensor("mask", [N, 128], dt.float32, kind="ExternalInput").ap()
    dmask_d = nc.dram_tensor("dmask", [N, 128], dt.bfloat16, kind="ExternalInput").ap()
    onesv_d = nc.dram_tensor("onesv", [128, 1], dt.bfloat16, kind="ExternalInput").ap()
    ones1_d = nc.dram_tensor("ones1", [1, 128], dt.bfloat16, kind="ExternalInput").ap()
    idn_d = nc.dram_tensor("idn", [N, N], dt.float32, kind="ExternalInput").ap()

    out_d = nc.dram_tensor("out", [BL, F_OUT], dt.float32, kind="ExternalOutput").ap()
    e2_d = nc.dram_tensor("e2", [ROWS, N], dt.float32, kind="ExternalOutput").ap()

    F32 = dt.float32
    B16 = dt.bfloat16

    with tile.TileContext(nc) as tc, ExitStack() as ctx:
        const = ctx.enter_context(tc.tile_pool(name="const", bufs=1))
        atp = ctx.enter_context(tc.tile_pool(name="atp", bufs=1))
        enp = ctx.enter_context(tc.tile_pool(name="enp", bufs=1))
        outp = ctx.enter_context(tc.tile_pool(name="outp", bufs=2))

        # ---- constants ----
        mask_t = const.tile([N, 128], F32, tag="mask")
        nc.sync.dma_start(mask_t[:], mask_d[:])
        dmask_t = const.tile([N, 128], B16, tag="dmask")
        nc.sync.dma_start(dmask_t[:], dmask_d[:])
        ones1_t = const.tile([1, 128], B16, tag="ones1")
        nc.sync.dma_start(ones1_t[:], ones1_d[:])
        idn_t = const.tile([N, N], F32, tag="idn")
        nc.sync.dma_start(idn_t[:], idn_d[:])

        # attn transposed, tile fs: [128 f, (i,b)] -> f' = i*1024 + fs*128 + p
        at_t = [atp.tile([128, N * BL], B16, tag=f"at{fs}", name=f"at{fs}")
                for fs in range(8)]
        if variant < 4:
            for fs in range(8):
                nc.vector.memset(at_t[fs][:], 0.0)

        # E (scaled, diag-boosted) per group, kept for the attn pass
        en_t = [enp.tile([N, 128], B16, tag=f"en{g}", name=f"en{g}")
                for g in range(NG)]

        # ============ mid scope: phase 0 + PASS A (needs xt/wg) ============
        with ExitStack() as mid:
            xtp = mid.enter_context(tc.tile_pool(name="xtp", bufs=1))
            wgp = mid.enter_context(tc.tile_pool(name="wgp", bufs=1))
            rnp = mid.enter_context(tc.tile_pool(name="rnp", bufs=1))

            rn_all = rnp.tile([1, 2 * ROWS], F32, tag="rnall")
            rn16 = rnp.tile([1, 2 * ROWS], B16, tag="rn16")

            xt_t = []
            for kc in range(8):
                t = xtp.tile([128, ROWS], B16, tag=f"xt{kc}", name=f"xt{kc}")
                nc.sync.dma_start(t[:], xt_d[kc * 128:(kc + 1) * 128, :])
                xt_t.append(t)

            wgT_t = [wgp.tile([128, ROWS], B16, tag=f"wg{ko}", name=f"wg{ko}")
                     for ko in range(8)]

            # ---------------- phase 0: wg + norms ----------------
            with ExitStack() as p0:
                w1p = p0.enter_context(tc.tile_pool(name="w1p", bufs=1))
                sb0 = p0.enter_context(tc.tile_pool(name="sb0", bufs=3))
                ps_wg = p0.enter_context(
                    tc.tile_pool(name="ps_wg", bufs=2, space="PSUM"))
                ps_n = p0.enter_context(
                    tc.tile_pool(name="ps_n", bufs=2, space="PSUM"))

                b1t_t = sb0.tile([128, 8], F32, tag="b1t", bufs=1)
                nc.sync.dma_start(b1t_t[:], b1t_d[:])
                onesv_t = sb0.tile([128, 1], B16, tag="onesv", bufs=1)
                nc.sync.dma_start(onesv_t[:], onesv_d[:])

                w1t_t = []
                for kc in range(8):
                    t = w1p.tile([128, F_IN], B16, tag=f"w1t{kc}",
                                 name=f"w1t{kc}")
                    nc.sync.dma_start(t[:], w1t_d[kc * 128:(kc + 1) * 128, :])
                    w1t_t.append(t)

                # x norms: |x_(b,i)|^2 = colsum over k of xt^2
                for rb in range(4):
                    sl = slice(rb * 512, (rb + 1) * 512)
                    pn = ps_n.tile([1, 512], F32, tag="pn")
                    for kc in range(8):
                        sq = sb0.tile([128, 512], B16, tag="sq")
                        nc.scalar.activation(sq[:], xt_t[kc][:, sl], AF.Square)
                        nc.tensor.matmul(pn[:], onesv_t[:], sq[:],
                                         start=(kc == 0), stop=(kc == 7))
                    s = sb0.tile([1, 512], F32, tag="s")
                    nc.scalar.sqrt(s[:], pn[:])
                    nc.vector.reciprocal(rn_all[0:1, sl], s[:])
                    nc.vector.tensor_copy(rn16[0:1, sl], rn_all[0:1, sl])

                # wg = W1 @ x^T + b1 (stored transposed) + wg norms
                for rb in range(4):
                    sl = slice(rb * 512, (rb + 1) * 512)
                    pnw = ps_n.tile([1, 512], F32, tag="pnw")
                    for ko in range(8):
                        pw = ps_wg.tile([128, 512], F32, tag="pw")
                        for ki in range(8):
                            nc.tensor.matmul(
                                pw[:],
                                w1t_t[ki][:, ko * 128:(ko + 1) * 128],
                                xt_t[ki][:, sl],
                                start=(ki == 0), stop=(ki == 7))
                        nc.vector.tensor_scalar_add(
                            wgT_t[ko][:, sl], pw[:], b1t_t[:, ko:ko + 1])
                        sqw = sb0.tile([128, 512], B16, tag="sqw")
                        nc.scalar.activation(sqw[:], pw[:], AF.Square,
                                             bias=b1t_t[:, ko:ko + 1])
                        nc.tensor.matmul(pnw[:], onesv_t[:], sqw[:],
                                         start=(ko == 0), stop=(ko == 7))
                    s = sb0.tile([1, 512], F32, tag="s")
                    nc.scalar.sqrt(s[:], pnw[:])
                    wsl0 = slice(ROWS + rb * 512, ROWS + (rb + 1) * 512)
                    nc.vector.reciprocal(rn_all[0:1, wsl0], s[:])
                    nc.vector.tensor_copy(rn16[0:1, wsl0], rn_all[0:1, wsl0])

            # ---------------- PASS A: scores / softmax / e2 ----------------
            with ExitStack() as pA:
                sb1 = pA.enter_context(tc.tile_pool(name="sb1", bufs=3))
                ps_sm = pA.enter_context(
                    tc.tile_pool(name="ps_sm", bufs=6, space="PSUM"))

                for g in range(NG if variant >= 2 else 0):
                    rsl = slice(g * 128, (g + 1) * 128)

                    # raw scores [i, (b, j)] = x_i . wg_j
                    pPN = ps_sm.tile([N, 128], F32, tag="sm", name="pPN",
                                     padded_shape=[128, 512])
                    for b in range(GB):
                        bs = slice(b * 16, (b + 1) * 16)
                        c = slice((g * GB + b) * 16, (g * GB + b + 1) * 16)
                        for kc in range(8):
                            nc.tensor.matmul(pPN[:, bs], xt_t[kc][:, c],
                                             wgT_t[kc][:, c],
                                             start=(kc == 0), stop=(kc == 7))

                    if variant == 2:
                        ets = sb1.tile([N, 128], F32, tag="ets", name="ets")
                        nc.vector.tensor_copy(ets[:], pPN[:])
                        nc.sync.dma_start(
                            e2_d[rsl, :].rearrange("(b i) j -> i b j", i=N),
                            ets[:].rearrange("p (b j) -> p b j", b=GB))
                        continue

                    # norm outer products [i, (b, j)] = rnx_i * rnwg_j
                    pM2 = ps_sm.tile([N, 128], F32, tag="sm", name="pM2",
                                     padded_shape=[128, 512])
                    for b in range(GB):
                        bs = slice(b * 16, (b + 1) * 16)
                        c = slice((g * GB + b) * 16, (g * GB + b + 1) * 16)
                        nc.tensor.matmul(pM2[:, bs], rn16[0:1, c],
                                         rn16[0:1, ROWS + c.start:ROWS + c.stop],
                                         start=True, stop=True)

                    # E = raw * rn_outer * mask  (natural layout [i, (b,j)])
                    m2s = sb1.tile([N, 128], F32, tag="m2s", name="m2s")
                    nc.vector.tensor_tensor(m2s[:], pM2[:], mask_t[:],
                                            op=ALU.mult)
                    enf = sb1.tile([N, 128], F32, tag="enf", name="enf")
                    nc.vector.tensor_tensor(enf[:], pPN[:], m2s[:],
                                            op=ALU.mult)
                    nc.vector.tensor_copy(en_t[g][:], enf[:])  # bf16 for attn

                    # transposed copy [j, (b, i)] via PE per-block transpose
                    pT = ps_sm.tile([N, 128], F32, tag="sm", name="pT",
                                    padded_shape=[128, 512])
                    for b in range(GB):
                        bs = slice(b * 16, (b + 1) * 16)
                        nc.tensor.transpose(pT[:, bs], enf[:, bs], idn_t[:])

                    # softmax over i (free segments of pT)
                    sexp = sb1.tile([N, 128], F32, tag="sexp", name="sexp")
                    nc.scalar.activation(sexp[:], pT[:], AF.Exp)
                    ssum = sb1.tile([N, GB], F32, tag="ssum", name="ssum")
                    nc.vector.tensor_reduce(
                        ssum[:], sexp[:].rearrange("p (b i) -> p b i", b=GB),
                        axis=mybir.AxisListType.X, op=ALU.add)
                    rs = sb1.tile([N, GB], F32, tag="rs", name="rs")
                    nc.vector.reciprocal(rs[:], ssum[:])
                    stb = sb1.tile([N, 128], B16, tag="stb", name="stb")
                    nc.vector.tensor_tensor(
                        stb[:].rearrange("p (b i) -> p b i", b=GB),
                        sexp[:].rearrange("p (b i) -> p b i", b=GB),
                        rs[:].broadcast_to([N, GB, N]), op=ALU.mult)

                    # message passing
                    mt = sb1.tile([N, 128], B16, tag="mt", name="mt")
                    nc.sync.dma_start(
                        mt[:].rearrange("p (b k) -> p b k", b=GB),
                        mat_d[rsl, :].rearrange("(b t) k -> t b k", t=N))
                    dscr = sb1.tile([N, 128], B16, tag="dscr", name="dscr")
                    nc.vector.tensor_tensor(dscr[:], mt[:], dmask_t[:],
                                            op=ALU.mult)
                    dv = sb1.tile([N, GB], F32, tag="dv", name="dv")
                    nc.vector.tensor_reduce(
                        dv[:], dscr[:].rearrange("p (b k) -> p b k", b=GB),
                        axis=mybir.AxisListType.X, op=ALU.add)

                    pFI = ps_sm.tile([N, 128], F32, tag="sm", name="pFI",
                                     padded_shape=[128, 512])
                    for b in range(GB):
                        bs = slice(b * 16, (b + 1) * 16)
                        nc.tensor.matmul(pFI[:, bs], mt[:, bs], stb[:, bs],
                                         start=True, stop=True)
                    fit = sb1.tile([N, 128], B16, tag="fit", name="fit")
                    nc.vector.tensor_copy(fit[:], pFI[:])
                    fjt = sb1.tile([N, 128], B16, tag="fjt", name="fjt")
                    nc.vector.tensor_tensor(
                        fjt[:].rearrange("p (b i) -> p b i", b=GB),
                        stb[:].rearrange("p (b i) -> p b i", b=GB),
                        dv[:].broadcast_to([N, GB, N]), op=ALU.mult)

                    pE2 = ps_sm.tile([N, 128], F32, tag="sm", name="pE2",
                                     padded_shape=[128, 512])
                    pE2r = ps_sm.tile([N, 128], F32, tag="sm", name="pE2r",
                                      padded_shape=[128, 512])
                    for b in range(GB):
                        bs = slice(b * 16, (b + 1) * 16)
                        nc.tensor.matmul(pE2[:, bs], fit[:, bs], fjt[:, bs],
                                         start=True, stop=True)
                        nc.tensor.matmul(pE2r[:, bs], fjt[:, bs], fit[:, bs],
                                         start=True, stop=True)
                    cpr = sb1.tile([N, 128], F32, tag="cpr", name="cpr")
                    nc.vector.tensor_scalar(cpr[:], pE2r[:], 0.5, None,
                                            op0=ALU.mult)
                    e2s = sb1.tile([N, 128], F32, tag="e2s", name="e2s")
                    nc.vector.scalar_tensor_tensor(
                        e2s[:], pE2[:], 0.5, cpr[:], op0=ALU.mult, op1=ALU.add)
                    nc.sync.dma_start(
                        e2_d[rsl, :].rearrange("(b i) j -> i b j", i=N),
                        e2s[:].rearrange("p (b j) -> p b j", b=GB))

        # ============ late scope: PASS B (attn) + phase 2 ============
        with ExitStack() as late:
            wmp = late.enter_context(tc.tile_pool(name="wmp", bufs=12))
            bmp = late.enter_context(tc.tile_pool(name="bmp", bufs=1))

            bm_t = bmp.tile([1, F_OUT], B16, tag="bm")
            nc.sync.dma_start(bm_t[:], bm_d[:])

            # PASS B: attn^T = tanh(X^T @ E)
            pB = late.enter_context(ExitStack())
            xnp = pB.enter_context(tc.tile_pool(name="xnp", bufs=2))
            ps_at = pB.enter_context(
                tc.tile_pool(name="ps_at", bufs=2, space="PSUM"))
            for g in range(NG if variant >= 4 else 0):
                rsl = slice(g * 128, (g + 1) * 128)
                xn16 = xnp.tile([N, GB * F_IN], B16, tag="xn16", name="xn16")
                nc.sync.dma_start(
                    xn16[:].rearrange("p (b f) -> p b f", b=GB),
                    xn_d[rsl, :].rearrange("(b r) f -> r b f", r=N))
                for fs in range(8):
                    pa = ps_at.tile([128, 128], F32, tag="pa", name="pa",
                                    padded_shape=[128, 512])
                    for b in range(GB):
                        bs = slice(b * 16, (b + 1) * 16)
                        nc.tensor.matmul(
                            pa[:, bs],
                            xn16[:, b * F_IN + fs * 128:
                                 b * F_IN + (fs + 1) * 128],
                            en_t[g][:, bs], start=True, stop=True)
                    src_ap = pa[:].rearrange("p (b i) -> p i b", b=GB)
                    dst_ap = at_t[fs][:].rearrange("p (i b) -> p i b", i=N)[
                        :, :, g * GB:(g + 1) * GB]
                    nc.scalar.activation(dst_ap, src_ap, AF.Tanh)

            pB.close()

            # phase 2: out = attn @ W_mlp^T + b_mlp  (fs-major chunk order)
            ps2 = late.enter_context(
                tc.tile_pool(name="ps2", bufs=1, space="PSUM"))
            pso = [ps2.tile([128, 512], F32, tag=f"o{oc}", name=f"pso{oc}")
                   for oc in range(8)]
            first = True
            for fs in range(8):
                for i_ in range(16):
                    fcg = i_ * 8 + fs
                    wm = wmp.tile([128, F_OUT], B16, tag="wm")
                    for h in range(4):
                        nc.sync.dma_start(
                            wm[:, h * 1024:(h + 1) * 1024],
                            wmt_d[fcg * 128:(fcg + 1) * 128,
                                  h * 1024:(h + 1) * 1024])
                    lhs = at_t[fs][:, i_ * 128:(i_ + 1) * 128]
                    for oc in range(8):
                        nc.tensor.matmul(pso[oc][:], lhs,
                                         wm[:, oc * 512:(oc + 1) * 512],
                                         start=first, stop=False)
                    first = False
            for oc in range(8):
                osl = slice(oc * 512, (oc + 1) * 512)
                nc.tensor.matmul(pso[oc][:], ones1_t[:], bm_t[0:1, osl],
                                 start=False, stop=True)
                ot = outp.tile([128, 512], F32, tag="ot")
                nc.vector.tensor_copy(ot[:], pso[oc][:])
                nc.sync.dma_start(out_d[:, osl], ot[:])

    nc.compile()
    return nc


def _get_program():
    variant = int(os.environ.get("KERNEL_VARIANT", "4"))
    key = f"nc{variant}"
    if key not in _CACHE:
        _CACHE[key] = _build_program(variant)
    return _CACHE[key]


def _prep_inputs(x, matrix, W1, b1, W_mlp, b_mlp, a_param):
    x = np.asarray(x, dtype=np.float32)
    matrix = np.asarray(matrix, dtype=np.float32)
    W1 = np.asarray(W1, dtype=np.float32)
    b1 = np.asarray(b1, dtype=np.float32)
    W_mlp = np.asarray(W_mlp, dtype=np.float32)
    b_mlp = np.asarray(b_mlp, dtype=np.float32)
    a = float(a_param)

    xr = x.reshape(NCORES, ROWS, F_IN)
    xn_h = xr.astype(BF16)
    xt_h = np.ascontiguousarray(xr.transpose(0, 2, 1)).astype(BF16)
    mat_h = matrix.reshape(NCORES, ROWS, N).astype(BF16)
    w1t_h = np.ascontiguousarray(W1.T).astype(BF16)
    b1t_h = np.ascontiguousarray(b1.reshape(8, 128).T)
    wmt_h = np.ascontiguousarray(W_mlp.astype(BF16).T)
    bm_h = b_mlp.reshape(1, F_OUT).astype(BF16)

    eye16 = np.eye(16, dtype=np.float32)
    eye_t = np.tile(eye16, (1, 8))                  # [16, 128]
    mask_h = (1.0 + a * eye_t).astype(np.float32)
    dmask_h = eye_t.astype(BF16)
    onesv_h = np.ones((128, 1), BF16)
    ones1_h = np.ones((1, 128), BF16)
    idn_h = np.eye(16, dtype=np.float32)

    in_maps = []
    for c in range(NCORES):
        in_maps.append({
            "xn": np.ascontiguousarray(xn_h[c]),
            "xt": np.ascontiguousarray(xt_h[c]),
            "mat": np.ascontiguousarray(mat_h[c]),
            "w1t": w1t_h, "b1t": b1t_h, "wmt": wmt_h, "bm": bm_h,
            "mask": mask_h, "dmask": dmask_h,
            "onesv": onesv_h, "ones1": ones1_h, "idn": idn_h,
        })
    return in_maps


def _install_ntff_hook():
    """Provide antenv.axon_hooks if the image lacks it (mirrors trn_boot)."""
    import types
    import ctypes
    import contextlib

    try:
        from antenv.axon_hooks import get_axon_ntff_profile_hook  # noqa: F401
        return
    except ImportError:
        pass

    so_path = "/opt/axon/libaxon_pjrt.so"
    if not os.path.exists(so_path):
        return
    lib = ctypes.CDLL(so_path)
    if not hasattr(lib, "axon_start_nrt_profile"):
        return
    lib.axon_start_nrt_profile.argtypes = [
        ctypes.POINTER(ctypes.c_int64), ctypes.c_size_t]
    lib.axon_start_nrt_profile.restype = ctypes.c_int64
    lib.axon_stop_nrt_profile.argtypes = [ctypes.c_char_p]
    lib.axon_stop_nrt_profile.restype = ctypes.c_int64

    @contextlib.contextmanager
    def _hook(output_dir, device_ids):
        import jax
        jax.devices()
        if device_ids:
            ids = (ctypes.c_int64 * len(device_ids))(*device_ids)
            rc = lib.axon_start_nrt_profile(ids, len(device_ids))
        else:
            rc = lib.axon_start_nrt_profile(None, 0)
        if rc != 0:
            raise RuntimeError(f"axon_start_nrt_profile rc={rc}")
        try:
            yield
        finally:
            n = lib.axon_stop_nrt_profile(str(output_dir).encode())
            print(f"profile: {n} file(s) written to {output_dir}",
                  file=sys.stderr)

    import antenv
    mod = types.ModuleType("antenv.axon_hooks")
    mod.get_axon_ntff_profile_hook = lambda: _hook
    mod.set_axon_ntff_profile_hook = lambda h: None
    sys.modules["antenv.axon_hooks"] = mod
    antenv.axon_hooks = mod


def kernel(x, matrix, W1, b1, W_mlp, b_mlp, a_param):
    global LAST_EXEC_NS
    from concourse import bass_utils

    nc = _get_program()
    in_maps = _prep_inputs(x, matrix, W1, b1, W_mlp, b_mlp, a_param)

    trace = os.environ.get("KERNEL_TRACE") == "1"
    if trace:
        _install_ntff_hook()
        try:
            res = bass_utils.run_bass_kernel_spmd(
                nc, in_maps, core_ids=list(range(NCORES)), trace=True,
                tmpdir=os.environ.get("KERNEL_TRACE_DIR"))
        except Exception as e:
            print(f"traced run failed ({e!r}); retrying untraced",
                  file=sys.stderr)
            res = bass_utils.run_bass_kernel_spmd(
                nc, in_maps, core_ids=list(range(NCORES)), trace=False)
    else:
        res = bass_utils.run_bass_kernel_spmd(
            nc, in_maps, core_ids=list(range(NCORES)), trace=False)
    LAST_EXEC_NS = res.exec_time_ns

    outs = res.results
    out_b = np.concatenate([np.asarray(outs[c]["out"]) for c in range(NCORES)], axis=0)
    e2 = np.concatenate([np.asarray(outs[c]["e2"]) for c in range(NCORES)], axis=0)
    e2 = np.ascontiguousarray(e2.reshape(B, N, N, 1))
    out_full = np.ascontiguousarray(
        np.broadcast_to(out_b[:, None, :], (B, N, F_OUT)))
    m = np.asarray(matrix, dtype=np.float32).reshape(B, N, N, 1)
    return out_full, e2, m
